# revision 1
# baseline (speedup 1.0000x reference)
"""Trainium2 Bass kernel for nn_Model_39676907883957 (dense_transformer).

Math (per batch element b, with S = D = N = 2048):
    q = Xq @ Wq^T + bq            # [S, D]
    kT = Wk @ Xk^T + bk[:, None]  # [D, S]  (k projected directly in transposed layout)
    v = Xv @ Wv^T + bv            # [S, D]
    scores[i, j] = sum_m q[m, i] * kT[m, j]          # q^T @ k^T
    attn = softmax_rows(scores)
    out[n, i] = sum_j v[j, n] * attn[i, j]           # == (attn @ v)^T

Sharding: data-parallel over batch, B=8 -> one batch element per NeuronCore.

Numerics: all matmuls run on the PE in fp16. The softmax-critical path
(q/k projections and scores) uses a hi/lo fp16 split with 3 cross products,
giving ~2e-5 absolute error on the logits (better than fp32 matmul, at
1 cycle/row vs fp32's 4).  v and attn@v run single-pass fp16 (the softmax
output is insensitive at the 1e-3 level there).
"""

import numpy as np

import concourse.bass as bass
import concourse.bacc as bacc
import concourse.tile as tile
import concourse.mybir as mybir
from concourse.bass_utils import run_bass_kernel_spmd

B, S, D = 8, 2048, 2048
N = 2048                 # S == D
KT = N // 128            # 16 contraction tiles
NCHUNK = N // 512        # 4 free-dim chunks of 512
F16 = mybir.dt.float16
F32 = mybir.dt.float32
F8 = mybir.dt.float8e5
DR = mybir.MatmulPerfMode.DoubleRow
AX = mybir.AxisListType.X
EXP = mybir.ActivationFunctionType.Exp

_compiled = {}


def _build():
    nc = bacc.Bacc("TRN2", target_bir_lowering=False, debug=False)

    # ExternalInputs (per core). x* are host-transposed activations [d, s].
    xqh = nc.dram_tensor("xqh", [N, N], F16, kind="ExternalInput").ap()
    xql = nc.dram_tensor("xql", [N, N], F16, kind="ExternalInput").ap()
    xkh = nc.dram_tensor("xkh", [N, N], F16, kind="ExternalInput").ap()
    xkl = nc.dram_tensor("xkl", [N, N], F16, kind="ExternalInput").ap()
    xv = nc.dram_tensor("xv", [N, N], F16, kind="ExternalInput").ap()
    # host-transposed weights [d, e], hi/lo fp16
    wqh = nc.dram_tensor("wqh", [N, N], F16, kind="ExternalInput").ap()
    wql = nc.dram_tensor("wql", [N, N], F16, kind="ExternalInput").ap()
    wkh = nc.dram_tensor("wkh", [N, N], F16, kind="ExternalInput").ap()
    wkl = nc.dram_tensor("wkl", [N, N], F16, kind="ExternalInput").ap()
    wv = nc.dram_tensor("wv", [N, N], F16, kind="ExternalInput").ap()
    # biases: bqb/bvb broadcast across partitions [128, N]; bkp partition-major [128, 16]
    bqb = nc.dram_tensor("bqb", [128, N], F32, kind="ExternalInput").ap()
    bkp = nc.dram_tensor("bkp", [128, KT], F32, kind="ExternalInput").ap()
    bvb = nc.dram_tensor("bvb", [128, N], F32, kind="ExternalInput").ap()

    out = nc.dram_tensor("out", [N, N], F32, kind="ExternalOutput").ap()

    with tile.TileContext(nc, pool_alloc_mode="queue") as tc:
        with tc.tile_pool(name="dram", bufs=1, space="DRAM") as dram:
            q_h = dram.tile([N, N], F16, tag="q_h")
            q_l = dram.tile([N, N], F16, tag="q_l")
            k_h = dram.tile([N, N], F16, tag="k_h")
            k_l = dram.tile([N, N], F16, tag="k_l")
            v_f = dram.tile([N, N], F16, tag="v_f")
            at_f = dram.tile([N, N], F16, tag="at_f")

            with tc.tile_pool(name="psum", bufs=8, space="PSUM") as psum:
                _proj_rows(nc, tc, psum, xqh, xql, wqh, wql, bqb, q_h, q_l)
                _proj_cols(nc, tc, psum, xkh, xkl, wkh, wkl, bkp, k_h, k_l)
                _proj_v(nc, tc, psum, xv, wv, bvb, v_f)
                _scores_softmax(nc, tc, psum, q_h, q_l, k_h, k_l, at_f)
                _attn_v(nc, tc, psum, at_f, v_f, out)

    nc.compile()
    return nc


def _load_kblock(nc, pool, dram_ap, col_blk, tag, dt=F16):
    """Load DRAM[:, col_blk*128 : +128] ([N, 128]) into one [128, N] SBUF tile
    whose slice [:, k*128:(k+1)*128] is contraction-tile k (partition = row%128)."""
    t = pool.tile([128, N], dt, tag=tag)
    src = dram_ap[:, col_blk * 128:(col_blk + 1) * 128].rearrange(
        "(t p) s -> p t s", p=128
    )
    dst = t[:].rearrange("p (t s) -> p t s", t=KT)
    nc.sync.dma_start(dst, src)
    return t


def _load_resident_pair(nc, pool, hi, lo, tag):
    """Load two [N, N] fp16 DRAM tensors as KT resident [128, N] row-block
    tiles each, emitting chunk-0 loads first (so the first matmul group's
    moving slices land early) on the SWDGE path (separate from the HWDGE
    streaming loads)."""
    hts = [pool.tile([128, N], F16, tag=f"{tag}h{k}", name=f"{tag}h{k}") for k in range(KT)]
    lts = [pool.tile([128, N], F16, tag=f"{tag}l{k}", name=f"{tag}l{k}") for k in range(KT)]
    for c in range(NCHUNK):
        cs = slice(c * 512, (c + 1) * 512)
        for k in range(KT):
            nc.gpsimd.dma_start(hts[k][:, cs], hi[k * 128:(k + 1) * 128, cs])
        for k in range(KT):
            nc.gpsimd.dma_start(lts[k][:, cs], lo[k * 128:(k + 1) * 128, cs])
    return hts, lts


def _load_resident_single(nc, pool, src, tag):
    ts = [pool.tile([128, N], F16, tag=f"{tag}{k}", name=f"{tag}{k}") for k in range(KT)]
    for c in range(NCHUNK):
        cs = slice(c * 512, (c + 1) * 512)
        for k in range(KT):
            nc.gpsimd.dma_start(ts[k][:, cs], src[k * 128:(k + 1) * 128, cs])
    return ts


def _proj_rows(nc, tc, psum, xh, xl, wh, wl, bias_bcast, out_h, out_l):
    """q-style projection: out[s, e] = sum_d X^T[d, s] * W^T[d, e] + bias[e].
    Stationary = activation k-blocks, moving = resident weights."""
    with (
        tc.tile_pool(name="pr_w", bufs=1) as wpool,
        tc.tile_pool(name="pr_x", bufs=2) as xpool,
        tc.tile_pool(name="pr_s", bufs=4) as spool,
        tc.tile_pool(name="pr_b", bufs=1) as bpool,
    ):
        bb = bpool.tile([128, N], F32, tag="bias")
        nc.sync.dma_start(bb[:], bias_bcast[:])
        wh_t, wl_t = _load_resident_pair(nc, wpool, wh, wl, "w")
        for s in range(KT):
            ah = _load_kblock(nc, xpool, xh, s, "ah")
            al = _load_kblock(nc, xpool, xl, s, "al")
            for c in range(NCHUNK):
                cs = slice(c * 512, (c + 1) * 512)
                ps = psum.tile([128, 512], F32)
                for k in range(KT):
                    nc.tensor.matmul(ps[:], ah[:, k * 128:(k + 1) * 128],
                                     wh_t[k][:, cs], start=(k == 0), stop=False)
                for k in range(KT):
                    nc.tensor.matmul(ps[:], al[:, k * 128:(k + 1) * 128],
                                     wh_t[k][:, cs], start=False, stop=False)
                for k in range(KT):
                    nc.tensor.matmul(ps[:], ah[:, k * 128:(k + 1) * 128],
                                     wl_t[k][:, cs], start=False, stop=(k == KT - 1))
                f32t = spool.tile([128, 512], F32, tag="f32")
                nc.vector.tensor_add(f32t[:], ps[:], bb[:, cs])
                h16 = spool.tile([128, 512], F16, tag="h16")
                nc.vector.tensor_copy(h16[:], f32t[:])
                l16 = spool.tile([128, 512], F16, tag="l16")
                nc.vector.tensor_sub(l16[:], f32t[:], h16[:])
                nc.sync.dma_start(out_h[s * 128:(s + 1) * 128, cs], h16[:])
                nc.sync.dma_start(out_l[s * 128:(s + 1) * 128, cs], l16[:])


def _proj_cols(nc, tc, psum, xh, xl, wh, wl, bias_part, out_h, out_l):
    """kT-style projection: out[e, s] = sum_d W^T[d, e] * X^T[d, s] + bias[e].
    Stationary = weight k-blocks, moving = resident activations."""
    with (
        tc.tile_pool(name="pc_x", bufs=1) as xpool,
        tc.tile_pool(name="pc_w", bufs=2) as wpool,
        tc.tile_pool(name="pc_s", bufs=4) as spool,
        tc.tile_pool(name="pc_b", bufs=1) as bpool,
    ):
        bp = bpool.tile([128, KT], F32, tag="biasp")
        nc.sync.dma_start(bp[:], bias_part[:])
        xh_t, xl_t = _load_resident_pair(nc, xpool, xh, xl, "x")
        for e in range(KT):
            gh = _load_kblock(nc, wpool, wh, e, "gh")
            gl = _load_kblock(nc, wpool, wl, e, "gl")
            for c in range(NCHUNK):
                cs = slice(c * 512, (c + 1) * 512)
                ps = psum.tile([128, 512], F32)
                for k in range(KT):
                    nc.tensor.matmul(ps[:], gh[:, k * 128:(k + 1) * 128],
                                     xh_t[k][:, cs], start=(k == 0), stop=False)
                for k in range(KT):
                    nc.tensor.matmul(ps[:], gh[:, k * 128:(k + 1) * 128],
                                     xl_t[k][:, cs], start=False, stop=False)
                for k in range(KT):
                    nc.tensor.matmul(ps[:], gl[:, k * 128:(k + 1) * 128],
                                     xh_t[k][:, cs], start=False, stop=(k == KT - 1))
                f32t = spool.tile([128, 512], F32, tag="f32")
                nc.vector.tensor_scalar_add(f32t[:], ps[:], bp[:, e:e + 1])
                h16 = spool.tile([128, 512], F16, tag="h16")
                nc.vector.tensor_copy(h16[:], f32t[:])
                l16 = spool.tile([128, 512], F16, tag="l16")
                nc.vector.tensor_sub(l16[:], f32t[:], h16[:])
                nc.sync.dma_start(out_h[e * 128:(e + 1) * 128, cs], h16[:])
                nc.sync.dma_start(out_l[e * 128:(e + 1) * 128, cs], l16[:])


def _proj_v(nc, tc, psum, xv, wv, bias_bcast, out_f):
    """v projection, single-pass fp16: out[s, e] = sum_d Xv^T[d, s]*Wv^T[d, e] + bv[e]."""
    with (
        tc.tile_pool(name="pv_w", bufs=1) as wpool,
        tc.tile_pool(name="pv_x", bufs=2) as xpool,
        tc.tile_pool(name="pv_s", bufs=4) as spool,
        tc.tile_pool(name="pv_b", bufs=1) as bpool,
    ):
        bb = bpool.tile([128, N], F32, tag="biasv")
        nc.sync.dma_start(bb[:], bias_bcast[:])
        wv_t = _load_resident_single(nc, wpool, wv, "wv")
        for s in range(KT):
            av = _load_kblock(nc, xpool, xv, s, "av")
            for c in range(NCHUNK):
                cs = slice(c * 512, (c + 1) * 512)
                ps = psum.tile([128, 512], F32)
                for k in range(KT):
                    nc.tensor.matmul(ps[:], av[:, k * 128:(k + 1) * 128],
                                     wv_t[k][:, cs], start=(k == 0), stop=(k == KT - 1))
                v16 = spool.tile([128, 512], F16, tag="v16")
                nc.vector.tensor_add(v16[:], ps[:], bb[:, cs])
                nc.sync.dma_start(out_f[s * 128:(s + 1) * 128, cs], v16[:])


def _scores_softmax(nc, tc, psum, q_h, q_l, k_h, k_l, at_f):
    """scores[i, j] = sum_m q[m, i]*kT[m, j]; row softmax; store attn fp16."""
    with (
        tc.tile_pool(name="sc_k", bufs=1) as kpool,
        tc.tile_pool(name="sc_q", bufs=2) as qpool,
        tc.tile_pool(name="sc_s", bufs=2) as spool,
        tc.tile_pool(name="sc_t", bufs=4) as tpool,
    ):
        kh_t, kl_t = _load_resident_pair(nc, kpool, k_h, k_l, "k")
        for i in range(KT):
            qh = _load_kblock(nc, qpool, q_h, i, "qh")
            ql = _load_kblock(nc, qpool, q_l, i, "ql")
            pss = []
            for c in range(NCHUNK):
                cs = slice(c * 512, (c + 1) * 512)
                ps = psum.tile([128, 512], F32)
                for k in range(KT):
                    nc.tensor.matmul(ps[:], qh[:, k * 128:(k + 1) * 128],
                                     kh_t[k][:, cs], start=(k == 0), stop=False)
                for k in range(KT):
                    nc.tensor.matmul(ps[:], ql[:, k * 128:(k + 1) * 128],
                                     kh_t[k][:, cs], start=False, stop=False)
                for k in range(KT):
                    nc.tensor.matmul(ps[:], qh[:, k * 128:(k + 1) * 128],
                                     kl_t[k][:, cs], start=False, stop=(k == KT - 1))
                pss.append(ps)
            # row stats over the full 2048-wide row
            m4 = tpool.tile([128, NCHUNK], F32, tag="m4")
            for c in range(NCHUNK):
                nc.vector.reduce_max(m4[:, c:c + 1], pss[c][:], axis=AX)
            mx = tpool.tile([128, 1], F32, tag="mx")
            nc.vector.reduce_max(mx[:], m4[:], axis=AX)
            negm = tpool.tile([128, 1], F32, tag="negm")
            nc.scalar.mul(negm[:], mx[:], -1.0)
            af32 = spool.tile([128, N], F32, tag="af32")
            sume = tpool.tile([128, NCHUNK], F32, tag="sume")
            for c in range(NCHUNK):
                cs = slice(c * 512, (c + 1) * 512)
                nc.scalar.activation(af32[:, cs], pss[c][:], EXP,
                                     bias=negm[:], scale=1.0,
                                     accum_out=sume[:, c:c + 1])
            tot = tpool.tile([128, 1], F32, tag="tot")
            nc.vector.reduce_sum(tot[:], sume[:], axis=AX)
            rcp = tpool.tile([128, 1], F32, tag="rcp")
            nc.vector.reciprocal(rcp[:], tot[:])
            a16 = spool.tile([128, N], F16, tag="a16")
            nc.vector.tensor_scalar_mul(a16[:], af32[:], rcp[:])
            nc.sync.dma_start(at_f[i * 128:(i + 1) * 128, :], a16[:])


def _attn_v(nc, tc, psum, at_f, v_f, out):
    """out[n, i] = sum_j v[j, n] * attn[i, j]; attn transposed via DMA xbar."""
    with (
        tc.tile_pool(name="av_t", bufs=1) as tpool,
        tc.tile_pool(name="av_v", bufs=2) as vpool,
        tc.tile_pool(name="av_s", bufs=4) as spool,
    ):
        att = []
        for j in range(KT):
            t = tpool.tile([128, N], F16, tag=f"at{j}")
            nc.sync.dma_start_transpose(t[:], at_f[:, j * 128:(j + 1) * 128])
            att.append(t)
        for n in range(KT):
            vt = _load_kblock(nc, vpool, v_f, n, "vt")
            for c in range(NCHUNK):
                cs = slice(c * 512, (c + 1) * 512)
                ps = psum.tile([128, 512], F32)
                for j in range(KT):
                    nc.tensor.matmul(ps[:], vt[:, j * 128:(j + 1) * 128],
                                     att[j][:, cs], start=(j == 0), stop=(j == KT - 1))
                o32 = spool.tile([128, 512], F32, tag="o32")
                nc.vector.tensor_copy(o32[:], ps[:])
                nc.sync.dma_start(out[n * 128:(n + 1) * 128, cs], o32[:])


def _split16(x):
    h = x.astype(np.float16)
    l = (x - h.astype(np.float32)).astype(np.float16)
    return h, l


def prepare_in_maps(query, key_, value, Wq, bq, Wk, bk, Wv, bv):
    query = np.asarray(query, dtype=np.float32)
    key_ = np.asarray(key_, dtype=np.float32)
    value = np.asarray(value, dtype=np.float32)
    Wq = np.asarray(Wq, dtype=np.float32)
    Wk = np.asarray(Wk, dtype=np.float32)
    Wv = np.asarray(Wv, dtype=np.float32)
    bq = np.asarray(bq, dtype=np.float32)
    bk = np.asarray(bk, dtype=np.float32)
    bv = np.asarray(bv, dtype=np.float32)

    wqh, wql = _split16(np.ascontiguousarray(Wq.T))
    wkh, wkl = _split16(np.ascontiguousarray(Wk.T))
    wvh = np.ascontiguousarray(Wv.T).astype(np.float16)
    bqb = np.broadcast_to(bq, (128, N)).copy()
    bvb = np.broadcast_to(bv, (128, N)).copy()
    bkp = np.ascontiguousarray(bk.reshape(KT, 128).T)

    in_maps = []
    for b in range(B):
        xqh, xql = _split16(np.ascontiguousarray(query[b].T))
        xkh, xkl = _split16(np.ascontiguousarray(key_[b].T))
        xvh = np.ascontiguousarray(value[b].T).astype(np.float16)
        in_maps.append({
            "xqh": xqh, "xql": xql, "xkh": xkh, "xkl": xkl, "xv": xvh,
            "wqh": wqh, "wql": wql, "wkh": wkh, "wkl": wkl, "wv": wvh,
            "bqb": bqb, "bkp": bkp, "bvb": bvb,
        })
    return in_maps


def get_nc():
    if "nc" not in _compiled:
        _compiled["nc"] = _build()
    return _compiled["nc"]


def kernel(query, key_, value, Wq, bq, Wk, bk, Wv, bv):
    in_maps = prepare_in_maps(query, key_, value, Wq, bq, Wk, bk, Wv, bv)
    res = run_bass_kernel_spmd(get_nc(), in_maps, core_ids=list(range(B)))
    return np.stack([res.results[b]["out"] for b in range(B)]).astype(np.float32)


if __name__ == "__main__":
    rng = np.random.default_rng(0)
    inputs = {
        "query": rng.standard_normal((B, S, D), dtype=np.float32),
        "key_": rng.standard_normal((B, S, D), dtype=np.float32),
        "value": rng.standard_normal((B, S, D), dtype=np.float32),
        "Wq": (rng.standard_normal((D, D), dtype=np.float32) / np.sqrt(D)),
        "bq": rng.standard_normal(D).astype(np.float32) * 0.01,
        "Wk": (rng.standard_normal((D, D), dtype=np.float32) / np.sqrt(D)),
        "bk": rng.standard_normal(D).astype(np.float32) * 0.01,
        "Wv": (rng.standard_normal((D, D), dtype=np.float32) / np.sqrt(D)),
        "bv": rng.standard_normal(D).astype(np.float32) * 0.01,
    }
    out = kernel(**inputs)
    print("out", out.shape, out.dtype)



# revision 2
# speedup vs baseline: 1.8918x; 1.8918x over previous
"""Trainium2 Bass kernel for nn_Model_39676907883957 (dense_transformer).

Math (per batch element b, with S = D = N = 2048):
    q = Xq @ Wq^T + bq            # [S, D]
    kT = Wk @ Xk^T + bk[:, None]  # [D, S]  (k projected directly in transposed layout)
    v = Xv @ Wv^T + bv            # [S, D]
    scores[i, j] = sum_m q[m, i] * kT[m, j]          # q^T @ k^T
    attn = softmax_rows(scores)
    out[n, i] = sum_j v[j, n] * attn[i, j]           # == (attn @ v)^T

Sharding: data-parallel over batch, B=8 -> one batch element per NeuronCore.

Numerics: ALL matmuls single-pass fp16 with fp32 PSUM accumulation
(5 matmul units of 2048^3). With scale_factor=1.0 the logits have
std ~ sqrt(2048) ~ 45, so the softmax is near-argmax per row and the
output tolerates fp16-level logit error; simulated rel err ~3.7e-3
against the fp32 reference (gate 2e-2).

Layout/dataflow per core:
  - q, v staged via DRAM fp16; kT written directly into SBUF-resident
    tiles by the k projection (no DRAM roundtrip).
  - attn row-blocks are transposed into SBUF-resident attn^T tiles via
    per-block [128,128] SBUF->SBUF DMA xbar transposes, overlapped with
    the scores/softmax loop; attn never touches DRAM.
  - attn@v streams v column-blocks as stationaries, moving = attn^T.
"""

import numpy as np

import concourse.bass as bass
import concourse.bacc as bacc
import concourse.tile as tile
import concourse.mybir as mybir
from concourse.bass_utils import run_bass_kernel_spmd

B, S, D = 8, 2048, 2048
N = 2048                 # S == D
KT = N // 128            # 16 contraction tiles
NCHUNK = N // 512        # 4 free-dim chunks of 512
F16 = mybir.dt.float16
F32 = mybir.dt.float32
AX = mybir.AxisListType.X
EXP = mybir.ActivationFunctionType.Exp

_compiled = {}


def _build():
    nc = bacc.Bacc("TRN2", target_bir_lowering=False, debug=False)

    # ExternalInputs (per core). x* are host-transposed activations [d, s].
    xq = nc.dram_tensor("xq", [N, N], F16, kind="ExternalInput").ap()
    xk = nc.dram_tensor("xk", [N, N], F16, kind="ExternalInput").ap()
    xv = nc.dram_tensor("xv", [N, N], F16, kind="ExternalInput").ap()
    # host-transposed weights [d, e]
    wq = nc.dram_tensor("wq", [N, N], F16, kind="ExternalInput").ap()
    wk = nc.dram_tensor("wk", [N, N], F16, kind="ExternalInput").ap()
    wv = nc.dram_tensor("wv", [N, N], F16, kind="ExternalInput").ap()
    # biases: bqb/bvb broadcast across partitions [128, N]; bkp partition-major [128, 16]
    bqb = nc.dram_tensor("bqb", [128, N], F32, kind="ExternalInput").ap()
    bkp = nc.dram_tensor("bkp", [128, KT], F32, kind="ExternalInput").ap()
    bvb = nc.dram_tensor("bvb", [128, N], F32, kind="ExternalInput").ap()

    out = nc.dram_tensor("out", [N, N], F32, kind="ExternalOutput").ap()

    with tile.TileContext(nc, pool_alloc_mode="queue") as tc:
        with tc.tile_pool(name="dram", bufs=1, space="DRAM") as dram:
            q_s = dram.tile([N, N], F16, tag="q_s")
            v_s = dram.tile([N, N], F16, tag="v_s")

            with tc.tile_pool(name="kr", bufs=1) as krp:
                kr = [krp.tile([128, N], F16, tag=f"kr{k}", name=f"kr{k}")
                      for k in range(KT)]
                with tc.tile_pool(name="ps_p", bufs=8, space="PSUM") as psum:
                    _proj_rows(nc, tc, psum, xq, wq, bqb, q_s)
                    _proj_cols_resident(nc, tc, psum, xk, wk, bkp, kr)
                    _proj_rows(nc, tc, psum, xv, wv, bvb, v_s)

                with tc.tile_pool(name="att", bufs=1) as attp:
                    att = [attp.tile([128, N], F16, tag=f"att{j}", name=f"att{j}")
                           for j in range(KT)]
                    _scores_softmax_t(nc, tc, q_s, kr, att)
                    _attn_v(nc, tc, v_s, att, out)

    nc.compile()
    return nc


def _load_kblock(nc, pool, dram_ap, col_blk, tag, dt=F16):
    """Load DRAM[:, col_blk*128 : +128] ([N, 128]) into one [128, N] SBUF tile
    whose slice [:, k*128:(k+1)*128] is contraction-tile k (partition = row%128)."""
    t = pool.tile([128, N], dt, tag=tag)
    src = dram_ap[:, col_blk * 128:(col_blk + 1) * 128].rearrange(
        "(t p) s -> p t s", p=128
    )
    dst = t[:].rearrange("p (t s) -> p t s", t=KT)
    nc.sync.dma_start(dst, src)
    return t


def _load_resident_single(nc, pool, src, tag):
    """Load an [N, N] fp16 DRAM tensor as KT resident [128, N] row-block tiles,
    chunk-0 loads first, on the SWDGE path."""
    ts = [pool.tile([128, N], F16, tag=f"{tag}{k}", name=f"{tag}{k}") for k in range(KT)]
    for c in range(NCHUNK):
        cs = slice(c * 512, (c + 1) * 512)
        for k in range(KT):
            nc.gpsimd.dma_start(ts[k][:, cs], src[k * 128:(k + 1) * 128, cs])
    return ts


def _proj_rows(nc, tc, psum, x, w, bias_bcast, out_dram):
    """q/v-style projection: out[s, e] = sum_d X^T[d, s] * W^T[d, e] + bias[e].
    Stationary = activation k-blocks, moving = resident weights. fp16 out to DRAM."""
    with (
        tc.tile_pool(name="pr_w", bufs=1) as wpool,
        tc.tile_pool(name="pr_x", bufs=2) as xpool,
        tc.tile_pool(name="pr_s", bufs=4) as spool,
        tc.tile_pool(name="pr_b", bufs=1) as bpool,
    ):
        bb = bpool.tile([128, N], F32, tag="bias")
        nc.sync.dma_start(bb[:], bias_bcast[:])
        wt = _load_resident_single(nc, wpool, w, "w")
        for s in range(KT):
            ax = _load_kblock(nc, xpool, x, s, "ax")
            for c in range(NCHUNK):
                cs = slice(c * 512, (c + 1) * 512)
                ps = psum.tile([128, 512], F32)
                for k in range(KT):
                    nc.tensor.matmul(ps[:], ax[:, k * 128:(k + 1) * 128],
                                     wt[k][:, cs], start=(k == 0), stop=(k == KT - 1))
                o16 = spool.tile([128, 512], F16, tag="o16")
                nc.vector.tensor_add(o16[:], ps[:], bb[:, cs])
                nc.sync.dma_start(out_dram[s * 128:(s + 1) * 128, cs], o16[:])


def _proj_cols_resident(nc, tc, psum, x, w, bias_part, kr):
    """kT projection: kr[e][p, s] = sum_d W^T[d, e*128+p] * X^T[d, s] + bk[e*128+p].
    Stationary = weight k-blocks, moving = resident activations. Output written
    directly into resident SBUF tiles (fp16)."""
    with (
        tc.tile_pool(name="pc_x", bufs=1) as xpool,
        tc.tile_pool(name="pc_w", bufs=2) as wpool,
        tc.tile_pool(name="pc_b", bufs=1) as bpool,
    ):
        bp = bpool.tile([128, KT], F32, tag="biasp")
        nc.sync.dma_start(bp[:], bias_part[:])
        xt = _load_resident_single(nc, xpool, x, "x")
        for e in range(KT):
            gw = _load_kblock(nc, wpool, w, e, "gw")
            for c in range(NCHUNK):
                cs = slice(c * 512, (c + 1) * 512)
                ps = psum.tile([128, 512], F32)
                for k in range(KT):
                    nc.tensor.matmul(ps[:], gw[:, k * 128:(k + 1) * 128],
                                     xt[k][:, cs], start=(k == 0), stop=(k == KT - 1))
                nc.vector.tensor_scalar_add(kr[e][:, cs], ps[:], bp[:, e:e + 1])


def _scores_softmax_t(nc, tc, q_s, kr, att):
    """scores[i, j] = sum_m q[m, i]*kT[m, j]; row softmax; transpose each
    [128,128] block of attn into resident attn^T tiles via the DMA xbar."""
    with (
        tc.tile_pool(name="sc_q", bufs=2) as qpool,
        tc.tile_pool(name="sc_e", bufs=2) as epool,
        tc.tile_pool(name="sc_a", bufs=4) as apool,
        tc.tile_pool(name="sc_t", bufs=4) as tpool,
        tc.tile_pool(name="ps_s", bufs=8, space="PSUM") as psum,
    ):
        for i in range(KT):
            qi = _load_kblock(nc, qpool, q_s, i, "qi")
            pss = []
            for c in range(NCHUNK):
                cs = slice(c * 512, (c + 1) * 512)
                ps = psum.tile([128, 512], F32)
                for k in range(KT):
                    nc.tensor.matmul(ps[:], qi[:, k * 128:(k + 1) * 128],
                                     kr[k][:, cs], start=(k == 0), stop=(k == KT - 1))
                pss.append(ps)
            # row stats over the full 2048-wide row
            m4 = tpool.tile([128, NCHUNK], F32, tag="m4")
            for c in range(NCHUNK):
                nc.vector.reduce_max(m4[:, c:c + 1], pss[c][:], axis=AX)
            mx = tpool.tile([128, 1], F32, tag="mx")
            nc.vector.reduce_max(mx[:], m4[:], axis=AX)
            negm = tpool.tile([128, 1], F32, tag="negm")
            nc.scalar.mul(negm[:], mx[:], -1.0)
            e16 = epool.tile([128, N], F16, tag="e16")
            sume = tpool.tile([128, NCHUNK], F32, tag="sume")
            for c in range(NCHUNK):
                cs = slice(c * 512, (c + 1) * 512)
                nc.scalar.activation(e16[:, cs], pss[c][:], EXP,
                                     bias=negm[:], scale=1.0,
                                     accum_out=sume[:, c:c + 1])
            tot = tpool.tile([128, 1], F32, tag="tot")
            nc.vector.reduce_sum(tot[:], sume[:], axis=AX)
            rcp = tpool.tile([128, 1], F32, tag="rcp")
            nc.vector.reciprocal(rcp[:], tot[:])
            a16 = apool.tile([128, N], F16, tag="a16")
            nc.vector.tensor_scalar_mul(a16[:], e16[:], rcp[:])
            # transpose the 16 [128,128] blocks of this attn row-block into
            # the resident attn^T tiles (SBUF -> SBUF via DMA xbar)
            iw = slice(i * 128, (i + 1) * 128)
            for j in range(KT):
                nc.scalar.dma_start_transpose(
                    att[j][:, iw], a16[:, j * 128:(j + 1) * 128])


def _attn_v(nc, tc, v_s, att, out):
    """out[n, i] = sum_j v[j, n] * attn[i, j]; stationary = v col-blocks
    (streamed), moving = resident attn^T tiles."""
    with (
        tc.tile_pool(name="av_v", bufs=2) as vpool,
        tc.tile_pool(name="av_s", bufs=4) as spool,
        tc.tile_pool(name="ps_a", bufs=8, space="PSUM") as psum,
    ):
        for n in range(KT):
            vn = _load_kblock(nc, vpool, v_s, n, "vn")
            for c in range(NCHUNK):
                cs = slice(c * 512, (c + 1) * 512)
                ps = psum.tile([128, 512], F32)
                for j in range(KT):
                    nc.tensor.matmul(ps[:], vn[:, j * 128:(j + 1) * 128],
                                     att[j][:, cs], start=(j == 0), stop=(j == KT - 1))
                o32 = spool.tile([128, 512], F32, tag="o32")
                nc.vector.tensor_copy(o32[:], ps[:])
                nc.sync.dma_start(out[n * 128:(n + 1) * 128, cs], o32[:])


def prepare_in_maps(query, key_, value, Wq, bq, Wk, bk, Wv, bv):
    query = np.asarray(query, dtype=np.float32)
    key_ = np.asarray(key_, dtype=np.float32)
    value = np.asarray(value, dtype=np.float32)
    Wq = np.asarray(Wq, dtype=np.float32)
    Wk = np.asarray(Wk, dtype=np.float32)
    Wv = np.asarray(Wv, dtype=np.float32)
    bq = np.asarray(bq, dtype=np.float32)
    bk = np.asarray(bk, dtype=np.float32)
    bv = np.asarray(bv, dtype=np.float32)

    wq16 = np.ascontiguousarray(Wq.T).astype(np.float16)
    wk16 = np.ascontiguousarray(Wk.T).astype(np.float16)
    wv16 = np.ascontiguousarray(Wv.T).astype(np.float16)
    bqb = np.broadcast_to(bq, (128, N)).copy()
    bvb = np.broadcast_to(bv, (128, N)).copy()
    bkp = np.ascontiguousarray(bk.reshape(KT, 128).T)

    in_maps = []
    for b in range(B):
        in_maps.append({
            "xq": np.ascontiguousarray(query[b].T).astype(np.float16),
            "xk": np.ascontiguousarray(key_[b].T).astype(np.float16),
            "xv": np.ascontiguousarray(value[b].T).astype(np.float16),
            "wq": wq16, "wk": wk16, "wv": wv16,
            "bqb": bqb, "bkp": bkp, "bvb": bvb,
        })
    return in_maps


def get_nc():
    if "nc" not in _compiled:
        _compiled["nc"] = _build()
    return _compiled["nc"]


def kernel(query, key_, value, Wq, bq, Wk, bk, Wv, bv):
    in_maps = prepare_in_maps(query, key_, value, Wq, bq, Wk, bk, Wv, bv)
    res = run_bass_kernel_spmd(get_nc(), in_maps, core_ids=list(range(B)))
    return np.stack([res.results[b]["out"] for b in range(B)]).astype(np.float32)


if __name__ == "__main__":
    rng = np.random.default_rng(0)
    inputs = {
        "query": rng.standard_normal((B, S, D), dtype=np.float32),
        "key_": rng.standard_normal((B, S, D), dtype=np.float32),
        "value": rng.standard_normal((B, S, D), dtype=np.float32),
        "Wq": (rng.standard_normal((D, D), dtype=np.float32) / np.sqrt(D)),
        "bq": rng.standard_normal(D).astype(np.float32) * 0.01,
        "Wk": (rng.standard_normal((D, D), dtype=np.float32) / np.sqrt(D)),
        "bk": rng.standard_normal(D).astype(np.float32) * 0.01,
        "Wv": (rng.standard_normal((D, D), dtype=np.float32) / np.sqrt(D)),
        "bv": rng.standard_normal(D).astype(np.float32) * 0.01,
    }
    out = kernel(**inputs)
    print("out", out.shape, out.dtype)


# revision 16
# speedup vs baseline: 2.2237x; 1.1754x over previous
"""Trainium2 Bass kernel for nn_Model_39676907883957 (dense_transformer).

Math (per batch element b, with S = D = N = 2048):
    q = Xq @ Wq^T + bq            # [S, D]
    kT = Wk @ Xk^T + bk[:, None]  # [D, S]  (k projected directly in transposed layout)
    v = Xv @ Wv^T + bv            # [S, D]
    scores[i, j] = sum_m q[m, i] * kT[m, j]          # q^T @ k^T
    attn = softmax_rows(scores)
    out[n, i] = sum_j v[j, n] * attn[i, j]           # == (attn @ v)^T

Sharding: data-parallel over batch, B=8 -> one batch element per NeuronCore.

Numerics: ALL matmuls single-pass fp16 with fp32 PSUM accumulation
(5 matmul units of 2048^3). With scale_factor=1.0 the logits have
std ~ sqrt(2048) ~ 45, so the softmax is near-argmax per row and the
output tolerates fp16-level logit error; simulated rel err ~3.7e-3
against the fp32 reference (gate 2e-2).

Layout/dataflow per core:
  - q, v staged via DRAM fp16; kT written directly into SBUF-resident
    tiles by the k projection (no DRAM roundtrip).
  - attn row-blocks are transposed into SBUF-resident attn^T tiles via
    per-block [128,128] SBUF->SBUF DMA xbar transposes, overlapped with
    the scores/softmax loop; attn never touches DRAM.
  - attn@v streams v column-blocks as stationaries, moving = attn^T.
"""

import numpy as np

import concourse.bass as bass
import concourse.bacc as bacc
import concourse.tile as tile
import concourse.mybir as mybir
from concourse.bass_utils import run_bass_kernel_spmd

B, S, D = 8, 2048, 2048
N = 2048                 # S == D
KT = N // 128            # 16 contraction tiles
NCHUNK = N // 512        # 4 free-dim chunks of 512
F16 = mybir.dt.float16
F32 = mybir.dt.float32
AX = mybir.AxisListType.X
EXP = mybir.ActivationFunctionType.Exp

_compiled = {}


def _build():
    nc = bacc.Bacc("TRN2", target_bir_lowering=False, debug=False)

    # ExternalInputs (per core). x* are host-transposed activations [d, s].
    xq = nc.dram_tensor("xq", [N, N], F16, kind="ExternalInput").ap()
    xk = nc.dram_tensor("xk", [N, N], F16, kind="ExternalInput").ap()
    xv = nc.dram_tensor("xv", [N, N], F16, kind="ExternalInput").ap()
    # host-transposed weights [d, e]
    wq = nc.dram_tensor("wq", [N, N], F16, kind="ExternalInput").ap()
    wk = nc.dram_tensor("wk", [N, N], F16, kind="ExternalInput").ap()
    wv = nc.dram_tensor("wv", [N, N], F16, kind="ExternalInput").ap()
    # biases: bqb/bvb broadcast across partitions [128, N]; bkp partition-major [128, 16]
    bqb = nc.dram_tensor("bqb", [128, N], F32, kind="ExternalInput").ap()
    bkp = nc.dram_tensor("bkp", [128, KT], F32, kind="ExternalInput").ap()
    bvb = nc.dram_tensor("bvb", [128, N], F32, kind="ExternalInput").ap()

    out = nc.dram_tensor("out", [N, N], F32, kind="ExternalOutput").ap()

    with tile.TileContext(nc, pool_alloc_mode="queue") as tc:
        with tc.tile_pool(name="dram", bufs=1, space="DRAM") as dram:
            q_s = dram.tile([N, N], F16, tag="q_s")
            v_s = dram.tile([N, N], F16, tag="v_s")

            with (
                tc.tile_pool(name="kr", bufs=1) as krp,
                tc.tile_pool(name="ps", bufs=8, space="PSUM") as psum,
            ):
                kr = [krp.tile([128, N], F16, tag=f"kr{k}", name=f"kr{k}")
                      for k in range(KT)]
                _proj_rows(nc, tc, psum, xq, wq, bqb, q_s, startup=True)
                _proj_cols_resident(nc, tc, psum, xk, wk, bkp, kr)
                _proj_rows(nc, tc, psum, xv, wv, bvb, v_s)

                with tc.tile_pool(name="att", bufs=1) as attp:
                    # attn^T resident: att[p, j_blk*N + i] = attn[i, j_blk*128+p]
                    att = attp.tile([128, KT * N], F16, tag="att", name="att")
                    _scores_softmax_t(nc, tc, psum, q_s, kr, att)
                    _attn_v(nc, tc, psum, v_s, att, out)

    nc.compile()
    return nc


def _load_kblock(nc, pool, dram_ap, col_blk, tag, dt=F16, engine=None):
    """Load DRAM[:, col_blk*128 : +128] ([N, 128]) into one [128, N] SBUF tile
    whose slice [:, k*128:(k+1)*128] is contraction-tile k (partition = row%128)."""
    t = pool.tile([128, N], dt, tag=tag)
    src = dram_ap[:, col_blk * 128:(col_blk + 1) * 128].rearrange(
        "(t p) s -> p t s", p=128
    )
    dst = t[:].rearrange("p (t s) -> p t s", t=KT)
    (engine or nc.sync).dma_start(dst, src)
    return t


def _load_resident_single(nc, pool, src, tag, startup=False):
    """Load an [N, N] fp16 DRAM tensor as KT resident [128, N] row-block tiles,
    chunk-0 loads first, on the SWDGE path. With startup=True the chunk-0
    loads are spread across three DMA queues so the very first matmul group
    isn't serialized behind 16 loads on one queue."""
    ts = [pool.tile([128, N], F16, tag=f"{tag}{k}", name=f"{tag}{k}") for k in range(KT)]
    engines = [nc.gpsimd, nc.sync, nc.scalar]
    for c in range(NCHUNK):
        cs = slice(c * 512, (c + 1) * 512)
        for k in range(KT):
            eng = engines[k % 3] if (startup and c == 0) else nc.gpsimd
            eng.dma_start(ts[k][:, cs], src[k * 128:(k + 1) * 128, cs])
    return ts


def _proj_rows(nc, tc, psum, x, w, bias_bcast, out_dram, startup=False):
    """q/v-style projection: out[s, e] = sum_d X^T[d, s] * W^T[d, e] + bias[e].
    Stationary = activation k-blocks (sync queue), moving = resident weights
    (gpsimd queue); stores go out on the scalar queue so loads never queue
    behind them."""
    with (
        tc.tile_pool(name="pr_w", bufs=1) as wpool,
        tc.tile_pool(name="pr_x", bufs=2) as xpool,
        tc.tile_pool(name="pr_s", bufs=4) as spool,
        tc.tile_pool(name="pr_b", bufs=1) as bpool,
    ):
        ax = _load_kblock(nc, xpool, x, 0, "ax")
        wt = _load_resident_single(nc, wpool, w, "w", startup=startup)
        bb = bpool.tile([128, N], F32, tag="bias")
        nc.scalar.dma_start(bb[:], bias_bcast[:])
        for s in range(KT):
            for c in range(NCHUNK):
                cs = slice(c * 512, (c + 1) * 512)
                ps = psum.tile([128, 512], F32)
                for k in range(KT):
                    nc.tensor.matmul(ps[:], ax[:, k * 128:(k + 1) * 128],
                                     wt[k][:, cs], start=(k == 0), stop=(k == KT - 1))
                o16 = spool.tile([128, 512], F16, tag="o16")
                nc.vector.tensor_add(o16[:], ps[:], bb[:, cs])
                nc.scalar.dma_start(out_dram[s * 128:(s + 1) * 128, cs], o16[:])
            if s + 1 < KT:
                ax = _load_kblock(nc, xpool, x, s + 1, "ax")


def _proj_cols_resident(nc, tc, psum, x, w, bias_part, kr):
    """kT projection: kr[e][p, s] = sum_d W^T[d, e*128+p] * X^T[d, s] + bk[e*128+p].
    Stationary = weight k-blocks, moving = resident activations. Output written
    directly into resident SBUF tiles (fp16)."""
    with (
        tc.tile_pool(name="pc_x", bufs=1) as xpool,
        tc.tile_pool(name="pc_w", bufs=2) as wpool,
        tc.tile_pool(name="pc_b", bufs=1) as bpool,
    ):
        bp = bpool.tile([128, KT], F32, tag="biasp")
        nc.scalar.dma_start(bp[:], bias_part[:])
        xt = _load_resident_single(nc, xpool, x, "x")
        for e in range(KT):
            gw = _load_kblock(nc, wpool, w, e, "gw")
            for c in range(NCHUNK):
                cs = slice(c * 512, (c + 1) * 512)
                ps = psum.tile([128, 512], F32)
                for k in range(KT):
                    nc.tensor.matmul(ps[:], gw[:, k * 128:(k + 1) * 128],
                                     xt[k][:, cs], start=(k == 0), stop=(k == KT - 1))
                nc.vector.tensor_scalar_add(kr[e][:, cs], ps[:], bp[:, e:e + 1])


def _scores_softmax_t(nc, tc, psum, q_s, kr, att):
    """scores[i, j] = sum_m q[m, i]*kT[m, j]; row softmax; transpose each
    attn row-block into the resident attn^T tile via one batched DMA xbar op."""
    with (
        tc.tile_pool(name="sc_q", bufs=2) as qpool,
        tc.tile_pool(name="sc_e", bufs=2) as epool,
        tc.tile_pool(name="sc_a", bufs=4) as apool,
        tc.tile_pool(name="sc_t", bufs=4) as tpool,
    ):
        for i in range(KT):
            qi = _load_kblock(nc, qpool, q_s, i, "qi")
            pss = []
            for c in range(NCHUNK):
                cs = slice(c * 512, (c + 1) * 512)
                ps = psum.tile([128, 512], F32)
                for k in range(KT):
                    nc.tensor.matmul(ps[:], qi[:, k * 128:(k + 1) * 128],
                                     kr[k][:, cs], start=(k == 0), stop=(k == KT - 1))
                pss.append(ps)
            # row stats over the full 2048-wide row
            m4 = tpool.tile([128, NCHUNK], F32, tag="m4")
            for c in range(NCHUNK):
                nc.vector.reduce_max(m4[:, c:c + 1], pss[c][:], axis=AX)
            mx = tpool.tile([128, 1], F32, tag="mx")
            nc.vector.reduce_max(mx[:], m4[:], axis=AX)
            negm = tpool.tile([128, 1], F32, tag="negm")
            nc.scalar.mul(negm[:], mx[:], -1.0)
            e16 = epool.tile([128, N], F16, tag="e16")
            sume = tpool.tile([128, NCHUNK], F32, tag="sume")
            for c in range(NCHUNK):
                cs = slice(c * 512, (c + 1) * 512)
                nc.scalar.activation(e16[:, cs], pss[c][:], EXP,
                                     bias=negm[:], scale=1.0,
                                     accum_out=sume[:, c:c + 1])
            tot = tpool.tile([128, 1], F32, tag="tot")
            nc.vector.reduce_sum(tot[:], sume[:], axis=AX)
            rcp = tpool.tile([128, 1], F32, tag="rcp")
            nc.vector.reciprocal(rcp[:], tot[:])
            a16 = apool.tile([128, N], F16, tag="a16")
            nc.vector.tensor_scalar_mul(a16[:], e16[:], rcp[:])
            # one batched xbar transpose per row-block: writes the i-th
            # 128-column window of all 16 attn^T block-rows
            # (out[p, t, f] = in[f, t*128 + p])
            dst = att[:].rearrange("p (t x) -> p t x", t=KT)[
                :, :, i * 128:(i + 1) * 128]
            nc.sync.dma_start_transpose(dst, a16[:])


def _attn_v(nc, tc, psum, v_s, att, out):
    """out[n, i] = sum_j v[j, n] * attn[i, j]; stationary = v col-blocks
    (streamed via SWDGE), moving = resident attn^T. Chunk c only depends on
    attn row-blocks 4c..4c+3, so the first groups overlap the scores tail."""
    with (
        tc.tile_pool(name="av_v", bufs=3) as vpool,
        tc.tile_pool(name="av_s", bufs=4) as spool,
    ):
        for n in range(KT):
            vn = _load_kblock(nc, vpool, v_s, n, "vn", engine=nc.gpsimd)
            for c in range(NCHUNK):
                cs = slice(c * 512, (c + 1) * 512)
                ps = psum.tile([128, 512], F32)
                for j in range(KT):
                    nc.tensor.matmul(ps[:], vn[:, j * 128:(j + 1) * 128],
                                     att[:, j * N + c * 512:j * N + (c + 1) * 512],
                                     start=(j == 0), stop=(j == KT - 1))
                o32 = spool.tile([128, 512], F32, tag="o32")
                nc.vector.tensor_copy(o32[:], ps[:])
                nc.scalar.dma_start(out[n * 128:(n + 1) * 128, cs], o32[:])


def prepare_in_maps(query, key_, value, Wq, bq, Wk, bk, Wv, bv):
    query = np.asarray(query, dtype=np.float32)
    key_ = np.asarray(key_, dtype=np.float32)
    value = np.asarray(value, dtype=np.float32)
    Wq = np.asarray(Wq, dtype=np.float32)
    Wk = np.asarray(Wk, dtype=np.float32)
    Wv = np.asarray(Wv, dtype=np.float32)
    bq = np.asarray(bq, dtype=np.float32)
    bk = np.asarray(bk, dtype=np.float32)
    bv = np.asarray(bv, dtype=np.float32)

    wq16 = np.ascontiguousarray(Wq.T).astype(np.float16)
    wk16 = np.ascontiguousarray(Wk.T).astype(np.float16)
    wv16 = np.ascontiguousarray(Wv.T).astype(np.float16)
    bqb = np.broadcast_to(bq, (128, N)).copy()
    bvb = np.broadcast_to(bv, (128, N)).copy()
    bkp = np.ascontiguousarray(bk.reshape(KT, 128).T)

    in_maps = []
    for b in range(B):
        in_maps.append({
            "xq": np.ascontiguousarray(query[b].T).astype(np.float16),
            "xk": np.ascontiguousarray(key_[b].T).astype(np.float16),
            "xv": np.ascontiguousarray(value[b].T).astype(np.float16),
            "wq": wq16, "wk": wk16, "wv": wv16,
            "bqb": bqb, "bkp": bkp, "bvb": bvb,
        })
    return in_maps


def get_nc():
    if "nc" not in _compiled:
        _compiled["nc"] = _build()
    return _compiled["nc"]


def kernel(query, key_, value, Wq, bq, Wk, bk, Wv, bv):
    in_maps = prepare_in_maps(query, key_, value, Wq, bq, Wk, bk, Wv, bv)
    res = run_bass_kernel_spmd(get_nc(), in_maps, core_ids=list(range(B)))
    return np.stack([res.results[b]["out"] for b in range(B)]).astype(np.float32)


if __name__ == "__main__":
    rng = np.random.default_rng(0)
    inputs = {
        "query": rng.standard_normal((B, S, D), dtype=np.float32),
        "key_": rng.standard_normal((B, S, D), dtype=np.float32),
        "value": rng.standard_normal((B, S, D), dtype=np.float32),
        "Wq": (rng.standard_normal((D, D), dtype=np.float32) / np.sqrt(D)),
        "bq": rng.standard_normal(D).astype(np.float32) * 0.01,
        "Wk": (rng.standard_normal((D, D), dtype=np.float32) / np.sqrt(D)),
        "bk": rng.standard_normal(D).astype(np.float32) * 0.01,
        "Wv": (rng.standard_normal((D, D), dtype=np.float32) / np.sqrt(D)),
        "bv": rng.standard_normal(D).astype(np.float32) * 0.01,
    }
    out = kernel(**inputs)
    print("out", out.shape, out.dtype)
